# revision 8
# baseline (speedup 1.0000x reference)
"""Trainium2 Bass kernel for nn_CausalMultimodal (gnn_message_passing).

Math (per batch row b, fully row-local so batch shards freely over 8 cores):
    mask[i,j]  = (matrix*(matrix>0.1))[i,j] > 0.1
    agg[b,d]   = (Z[b,:] @ mask[d,:]) / count[d]   (0 when count==0)
    hidden     = relu(Z[b,d]*W1[d,0,h] + agg[b,d]*W1[d,1,h] + b1[d,h])
    E[b,d]     = sum_h hidden[b,d,h]*W2[d,h] + b2[d]

Since agg = Z @ M2 with M2[j,d] = mask[d,j]/count[d], the first layer folds
into one 32x128 matrix A computed host-side: U[b, 32h+d] = (Z @ A)[b, 32h+d];
then E = W2sel.T @ relu(U + b1) + b2 with W2sel (128,32) block-sparse.

v4 dataflow (PSUM-evacuation-bound; ACT+DVE are the critical engines):
  - Host pre-permutes Z (bf16) into the exact strip layout mm1 streams, and
    un-permutes the strip-layout E output. No DVE transposes on device.
  - Per 2048-row block: 4 row-tiled concurrent MMs (K=32, tile_position
    (32a,0)) write U into FOUR (128,512) strip tiles from a 6-deep 1-bank
    ring. Strip relus alternate engines (ACT: s0,s2 / DVE: s1,s3), so the
    slot-reuse WAR chain only passes through a single ~620ns strip op:
    strips 0-1 of block i+1 reuse slots read by strips 2-3 of block i-1
    (2-period slack) and strips 2-3 reuse slots of strips 0-1 of block i
    (the relus that run FIRST on each engine). 4 col-tiled concurrent MMs
    (M=32, tile_position (0,32a)) write E into a separate (128,512) eps
    tile; E is evacuated PSUM->SBUF bf16 on alternating engines and DMA'd
    per megatile.
  - PSUM budget: 6 strip-slots + 2 eps banks = 8 banks. The wall time
    tracks the busy-bound of the ACT/DVE engines (~1570ns/block).
  - Emission is software-pipelined: mm1 of block i+1 enters the PE FIFO
    before mm3 of block i; E-evac of block i is emitted one iteration late
    so it never head-of-line-blocks a relu in the strict-FIFO ACT/DVE
    queues.
"""

import os

import ml_dtypes
import numpy as np

import concourse.bacc as bacc
import concourse.tile as tile
from concourse import mybir
from concourse import bass_utils

B_TOTAL, D, H = 1048576, 32, 4
NCORES = 8
R = B_TOTAL // NCORES        # rows per core (131072)
NMT = 16                     # megatiles per core
BLOCKS_PER_MT = 4
NBLK = NMT * BLOCKS_PER_MT   # 64 blocks of 2048 rows
BF16 = ml_dtypes.bfloat16

EEVAC = os.environ.get("NNK_EEVAC", "alt")  # alt | act | dve
EOUT = os.environ.get("NNK_EOUT", "bf16")   # bf16 | f32
ZP_BUFS = int(os.environ.get("NNK_ZPBUFS", "3"))
UG_BUFS = int(os.environ.get("NNK_UGBUFS", "6"))
EPS_BUFS = int(os.environ.get("NNK_EPSBUFS", "2"))

_module_cache = {}


def _build_module(b1_zero, b2_zero):
    key = (b1_zero, b2_zero, EEVAC, EOUT, ZP_BUFS, UG_BUFS, EPS_BUFS)
    if key in _module_cache:
        return _module_cache[key]

    f32 = mybir.dt.float32
    bf = mybir.dt.bfloat16
    edt = bf if EOUT == "bf16" else f32

    nc = bacc.Bacc("TRN2", target_bir_lowering=False, debug=False,
                   num_devices=NCORES)

    ZP = nc.dram_tensor("ZP", (NMT, 128, 2048), bf, kind="ExternalInput").ap()
    A4 = nc.dram_tensor("A4", (128, 128), bf, kind="ExternalInput").ap()
    W2 = nc.dram_tensor("W2", (128, D), bf, kind="ExternalInput").ap()
    B1V = nc.dram_tensor("B1V", (128, 1), f32, kind="ExternalInput").ap()
    B2V = nc.dram_tensor("B2V", (128, 1), f32, kind="ExternalInput").ap()
    EP = nc.dram_tensor("EP", (NMT, 128, 2048), edt, kind="ExternalOutput").ap()

    with tile.TileContext(nc) as tc:
        with (
            tc.tile_pool(name="const", bufs=1) as constp,
            tc.tile_pool(name="zp", bufs=ZP_BUFS) as zpp,
            tc.tile_pool(name="vt", bufs=2) as vtp,
            tc.tile_pool(name="et", bufs=2) as etp,
            tc.tile_pool(name="ug", bufs=UG_BUFS, space="PSUM") as ugp,
            tc.tile_pool(name="ep", bufs=EPS_BUFS, space="PSUM") as epp,
        ):
            acst = constp.tile([128, 128], bf, name="cA4")
            nc.sync.dma_start(out=acst, in_=A4)
            wcst = constp.tile([128, D], bf, name="cW2")
            nc.sync.dma_start(out=wcst, in_=W2)
            b1v = constp.tile([128, 1], f32, name="cB1")
            nc.sync.dma_start(out=b1v, in_=B1V)
            b2v = constp.tile([128, 1], f32, name="cB2")
            nc.sync.dma_start(out=b2v, in_=B2V)

            zpt = {}    # megatile -> SBUF tile
            ugs = {}    # block -> (pair01, pair23) PSUM tiles
            vts = {}    # block -> (vt01, vt23) SBUF tiles
            eps_ = {}   # block -> eps PSUM tile
            ets = {}    # megatile -> SBUF E tile

            def fetch_zp(m):
                t = zpp.tile([128, 2048], bf, tag="zp", name=f"zp{m}")
                nc.sync.dma_start(out=t, in_=ZP[m])
                zpt[m] = t

            def mm1(i):
                m, t = divmod(i, BLOCKS_PER_MT)
                if t == 0 and m + ZP_BUFS - 1 < NMT:
                    fetch_zp(m + ZP_BUFS - 1)
                strips = [ugp.tile([128, 512], f32, tag="ug", name=f"u{a}")
                          for a in range(4)]
                z = zpt[m]
                for a in range(4):
                    nc.tensor.matmul(
                        strips[a],
                        lhsT=acst[32 * a:32 * (a + 1), :],
                        rhs=z[32 * a:32 * (a + 1), 512 * t:512 * (t + 1)],
                        start=True, stop=True,
                        tile_position=(32 * a, 0),
                    )
                ugs[i] = strips
                if t == BLOCKS_PER_MT - 1:
                    del zpt[m]

            def relu(i):
                strips = ugs[i]
                vs = [vtp.tile([128, 512], bf, tag=f"v{a}", name=f"v{a}")
                      for a in range(4)]
                for a in (0, 1, 2, 3):  # s0/s1 first on each engine
                    on_act = a % 2 == 0  # ACT: s0,s2; DVE: s1,s3
                    if b1_zero:
                        if on_act:
                            nc.scalar.activation(
                                vs[a], strips[a],
                                mybir.ActivationFunctionType.Relu)
                        else:
                            nc.vector.tensor_scalar_max(
                                vs[a], strips[a], 0.0)
                    else:
                        if on_act:
                            nc.scalar.activation(
                                vs[a], strips[a],
                                mybir.ActivationFunctionType.Relu,
                                bias=b1v, scale=1.0)
                        else:
                            nc.vector.tensor_scalar(
                                vs[a], strips[a], b1v, 0.0,
                                mybir.AluOpType.add, mybir.AluOpType.max)
                vts[i] = vs

            def mm3(i):
                vs = vts[i]
                ep = epp.tile([128, 512], f32, tag="ep", name="ep")
                for a in range(4):
                    nc.tensor.matmul(
                        ep[32 * a:32 * (a + 1), :],
                        lhsT=wcst,
                        rhs=vs[a],
                        start=True, stop=True,
                        tile_position=(0, 32 * a),
                    )
                eps_[i] = ep
                del ugs[i], vts[i]

            def eevac(i):
                m, t = divmod(i, BLOCKS_PER_MT)
                if t == 0:
                    ets[m] = etp.tile([128, 2048], edt, tag="et", name="et")
                et = ets[m]
                dst = et[:, 512 * t:512 * (t + 1)]
                src = eps_[i]
                if EEVAC == "act" or (EEVAC == "alt" and i % 2 == 0):
                    if b2_zero:
                        nc.scalar.activation(
                            dst, src, mybir.ActivationFunctionType.Identity)
                    else:
                        nc.scalar.activation(
                            dst, src, mybir.ActivationFunctionType.Identity,
                            bias=b2v, scale=1.0)
                else:
                    if b2_zero:
                        nc.vector.tensor_copy(dst, src)
                    else:
                        nc.vector.tensor_scalar_add(dst, src, b2v)
                del eps_[i]
                if t == BLOCKS_PER_MT - 1:
                    nc.sync.dma_start(out=EP[m], in_=et)
                    del ets[m]

            for m in range(min(ZP_BUFS - 1, NMT)):
                fetch_zp(m)
            mm1(0)
            for i in range(NBLK):
                if i + 1 < NBLK:
                    mm1(i + 1)
                relu(i)
                mm3(i)
                if i > 0:
                    eevac(i - 1)
            eevac(NBLK - 1)

    nc.compile()
    _module_cache[key] = nc
    return nc


def _fold_params(matrix, W1, b1, W2, b2):
    """Host-side fold of the tiny params into A4/W2S/B1V/B2V (a few KB)."""
    matrix = np.asarray(matrix, np.float32)
    W1 = np.asarray(W1, np.float32)
    b1 = np.asarray(b1, np.float32)
    W2 = np.asarray(W2, np.float32)
    b2 = np.asarray(b2, np.float32)

    alpha_est = matrix * (matrix > np.float32(0.1)).astype(np.float32)
    mask = (alpha_est > np.float32(0.1)).astype(np.float32)  # (D, D)
    cnt = mask.sum(axis=1)  # (D,)
    scale = np.where(cnt > 0, np.float32(1.0) / np.maximum(cnt, 1.0),
                     np.float32(0.0)).astype(np.float32)
    M2 = (mask.T * scale[None, :]).astype(np.float32)  # M2[j,d]

    A = np.zeros((D, D * H), np.float32)
    for h in range(H):
        Ah = M2 * W1[None, :, 1, h]  # (j, d): M2[j,d] * W1[d,1,h]
        Ah[np.arange(D), np.arange(D)] += W1[:, 0, h]
        A[:, D * h:D * (h + 1)] = Ah
    A4 = np.ascontiguousarray(np.tile(A, (4, 1)))  # (128, 128)

    W2S = np.zeros((D * H, D), np.float32)
    W2S[np.arange(D * H), np.tile(np.arange(D), H)] = W2.T.reshape(-1)
    B1V = np.ascontiguousarray(b1.T.reshape(D * H, 1))
    B2V = np.ascontiguousarray(np.tile(b2, H).reshape(D * H, 1))
    return A4, W2S, B1V, B2V, not np.any(b1), not np.any(b2)


def _pack_z(Z):
    """(B, 32) f32 -> per-core (NMT, 128, 2048) bf16 strip layout:
    ZP[c][m, 32a+j, 512t+cc] = Z[c*R + m*8192 + t*2048 + a*512 + cc, j]."""
    Zb = np.asarray(Z, np.float32).astype(BF16)
    v = Zb.reshape(NCORES, NMT, 4, 4, 512, D)      # [c, m, t, a, cc, j]
    v = v.transpose(0, 1, 3, 5, 2, 4)              # [c, m, a, j, t, cc]
    return np.ascontiguousarray(v).reshape(NCORES, NMT, 128, 2048)


def _unpack_e(EPs):
    """per-core (NMT, 128, 2048) strip layout -> (B, 32) f32."""
    v = np.stack([np.asarray(e) for e in EPs])     # [c, m, 128, 2048]
    v = v.reshape(NCORES, NMT, 4, D, 4, 512)       # [c, m, a, d, t, cc]
    v = v.transpose(0, 1, 4, 2, 5, 3)              # [c, m, t, a, cc, d]
    return np.ascontiguousarray(v).reshape(B_TOTAL, D).astype(np.float32)


def _run(Z, matrix, W1, b1, W2, b2, trace=False):
    Z = np.asarray(Z, np.float32)
    assert Z.shape == (B_TOTAL, D), Z.shape
    A4, W2S, B1V, B2V, b1_zero, b2_zero = _fold_params(matrix, W1, b1, W2, b2)
    nc = _build_module(b1_zero, b2_zero)

    ZPall = _pack_z(Z)
    cst = {
        "A4": np.ascontiguousarray(A4.astype(BF16)),
        "W2": np.ascontiguousarray(W2S.astype(BF16)),
        "B1V": B1V, "B2V": B2V,
    }
    in_maps = [{**cst, "ZP": ZPall[c]} for c in range(NCORES)]
    res = bass_utils.run_bass_kernel_spmd(
        nc, in_maps, core_ids=list(range(NCORES)), trace=trace)
    out = _unpack_e([r["EP"] for r in res.results])
    return out, res


def kernel(Z, matrix, W1, b1, W2, b2):
    out, _ = _run(Z, matrix, W1, b1, W2, b2, trace=False)
    return out
